# revision 16
# baseline (speedup 1.0000x reference)
"""Trainium2 Bass kernel for the LIF/hh neuron module.

Math (from the reference):
  fc = x @ W_fc.T + b_fc                    [B, T, C]
  per step t (state mem[B,C,4], spike[B,C]):
    x4   = mem[...,:3] @ w + b              (old mem)
    keep = DECAY * (1 - spike)
    mem03' = mem[...,:3]*keep + fc_t        (channels 0..2 identical updates!)
    mem3'  = mem[...,3]*keep + x4
    mem1 = mem03' @ w + b + mem3'
    spike' = mem1 > THRESH

Key identity: channels 0..2 of mem start at 0 and receive identical updates,
so m0==m1==m2 =: m for all t.  Let W = w0+w1+w2, u := W*m + b.  Then with
v_t := W * fc_t (folded into the GEMM weights on host), and b==0:
    u'    = DECAY*(1-s)*u + v_t
    m3'   = DECAY*(1-s)*m3 + u
    mem1  = u' + m3'
    s'    = mem1 > THRESH
Further, mem1 = u + m3 after update, so with n := 1-s (inverted spike):
    w_t   = u + v_t                       (off critical chain)
    mem1' = DECAY*(mem1*n) + w_t          (3-op critical chain with n' below)
    u'    = DECAY*(u*n) + v_t             (off chain)
    n'    = (mem1' <= THRESH)
State: (u, mem1, n).  Verified bit-identical to the reference recurrence.

GEMM: single fp32r (FP22) pass.  The tensor engine runs fp32r at 1 cycle/row
for moving dim >= 256 (vs 3 bf16 hi/lo passes for the same accuracy class),
and fp22's 13-bit mantissa gives ~2^-13 relative error -> ~200 spike flips
(rel ~6e-3, gate 2e-2).

Sharding: data-parallel over batch, B=256 -> 32 per core on 8 cores.
Per-core GEMM: [C=4096, K=4096] x [K, N=480] with N = t*32+b.
Recurrence layout: partition p = c%128, free = j*32 + b (j = c//128), G=4
column groups; group g's recurrence overlaps the GEMM of later groups'
M-tiles, with chain ops on Vector and off-chain ops on GpSimd.
"""
import sys
import os

sys.path.insert(0, "/opt/trn_rl_repo")

import numpy as np
import ml_dtypes

THRESH = 0.8
DECAY = 0.2

B, T, IN, C = 256, 15, 4096, 4096
NCORES = 8
BL = B // NCORES          # 32 batch per core
N = BL * T                # 480 moving columns per core
KS = IN // 128            # 32 K-subtiles
JC = C // 128             # 32 c-chunks (M-tiles)
FREE = JC * BL            # 1024 state free dim
GROUPS = (8, 8, 8, 4, 4)          # recurrence group widths in j-chunks
XCH = 8                   # x load chunks (startup pipelining)
KCH = KS // XCH           # K-subtiles per x chunk

LAST_EXEC_S = None
LAST_NC = None            # stashed Bass module for test harness profiling


def _numpy_fallback(x, W_fc, b_fc, W_lif, b_lif):
    fc = np.einsum("bti,ci->btc", x.astype(np.float64), W_fc.astype(np.float64))
    fc += b_fc.astype(np.float64)
    w = W_lif[0].astype(np.float64)
    b = float(b_lif[0])
    Bs, Ts, Cs = fc.shape
    mem = np.zeros((Bs, Cs, 4))
    spike = np.zeros((Bs, Cs))
    outs = []
    for t in range(Ts):
        x4 = mem[..., :3] @ w + b
        keep = DECAY * (1.0 - spike)
        mem03 = mem[..., :3] * keep[..., None] + fc[:, t][..., None]
        mem3 = mem[..., 3] * keep + x4
        mem = np.concatenate([mem03, mem3[..., None]], axis=-1)
        mem1 = mem03 @ w + b + mem3
        spike = (mem1 > THRESH).astype(np.float64)
        outs.append(spike)
    return np.stack(outs, axis=1).astype(x.dtype)


def _legalize_waits(nc, mybir):
    """Walrus codegen caps embedded sync-waits per instruction (Matmult: 1,
    DMACopy: 2, ...).  Tile's sem assignment can exceed that.  Engines and
    DMA sequencers execute their queues in order, so moving excess waits onto
    freshly inserted same-engine NoOps directly before the instruction is
    semantically identical.  One wait per NoOp (NoOp capacity unknown)."""
    limits = {}
    counter = [0]
    for fn in nc.m.functions:
        for blk in fn.blocks:
            insts = blk.instructions
            out = []
            changed = False
            for inst in insts:
                tname = type(inst).__name__
                lim = limits.get(tname, 1)
                si = inst.sync_info
                waits = list(si.on_wait) if si is not None else []
                if len(waits) > lim:
                    excess, kept = waits[:-lim], waits[-lim:]
                    for w in excess:
                        counter[0] += 1
                        out.append(mybir.InstNoOp(
                            name=f"WSPLIT-{counter[0]}",
                            engine=inst.engine,
                            ins=[], outs=[],
                            sync_info=mybir.SyncInfo(on_wait=[w], on_update=[]),
                        ))
                    inst.sync_info = mybir.SyncInfo(
                        on_wait=kept, on_update=list(si.on_update))
                    changed = True
                out.append(inst)
            if changed:
                blk.instructions = out
    return counter[0]


def _build_bass():
    import concourse.bass as bass
    import concourse.mybir as mybir
    import concourse.tile as tile
    from contextlib import ExitStack

    f32 = mybir.dt.float32
    f32r = mybir.dt.float32r
    Alu = mybir.AluOpType

    nc = bass.Bass()
    wt_d = nc.dram_tensor("wt", [JC, 128, KS, 128], f32r, kind="ExternalInput")
    xt_d = nc.dram_tensor("xt", [128, KS, N], f32r, kind="ExternalInput")
    sp_d = nc.dram_tensor("sp", [T, 128, FREE], f32, kind="ExternalOutput")

    with ExitStack() as ctx:
        tc = ctx.enter_context(tile.TileContext(nc))
        wpool = ctx.enter_context(tc.tile_pool(name="wpool", bufs=3))
        xpool = ctx.enter_context(tc.tile_pool(name="xpool", bufs=1))
        fcpool = ctx.enter_context(tc.tile_pool(name="fcpool", bufs=1))
        spool = ctx.enter_context(tc.tile_pool(name="state", bufs=1))
        ppool = ctx.enter_context(tc.tile_pool(name="psum", bufs=4, space="PSUM"))

        # x resident in SBUF, loaded in XCH chunks so the first matmuls
        # only wait on chunk 0; weight tiles stream per-M-tile (bufs=3
        # self-throttles the prefetch depth)
        # x loads and spike stores go through the Scalar engine's HWDGE
        # queue so the weight stream has the Sync queue to itself
        x_tiles = []
        for ci in range(XCH):
            xtile = xpool.tile([128, KCH, N], f32r, tag=f"x{ci}", name=f"x{ci}")
            nc.scalar.dma_start(xtile[:], xt_d[:, ci * KCH:(ci + 1) * KCH, :])
            x_tiles.append(xtile)

        fc_sbuf = fcpool.tile([128, T, FREE], f32)

        # per-group state tiles (distinct tags so group chains can overlap);
        # u and ns are double-buffered to avoid cross-engine WAR stalls
        st = []
        for g in range(len(GROUPS)):
            gf = GROUPS[g] * BL
            d = {nm: spool.tile([128, gf], f32, tag=f"{nm}{g}", name=f"{nm}{g}")
                 for nm in ("u", "uz", "w", "mem1", "z", "ns")}
            st.append(d)

        def emit_recurrence(g, j0):
            gf = GROUPS[g] * BL
            sl = slice(j0 * BL, j0 * BL + gf)
            d = st[g]
            u, uz, w = d["u"], d["uz"], d["w"]
            mem1, z, ns = d["mem1"], d["z"], d["ns"]
            nc.gpsimd.memset(u[:], 0.0)
            nc.vector.memset(mem1[:], 0.0)
            nc.vector.memset(ns[:], 1.0)
            for t in range(T):
                v_t = fc_sbuf[:, t, sl]
                # off-chain (GpSimd): w = u + v_t ; uz = u*ns
                nc.gpsimd.tensor_tensor(w[:], u[:], v_t, Alu.add)
                nc.gpsimd.tensor_tensor(uz[:], u[:], ns[:], Alu.mult)
                # chain (Vector): z = mem1*ns ; mem1' = D*z + w ; ns = mem1'<=th
                nc.vector.tensor_tensor(z[:], mem1[:], ns[:], Alu.mult)
                nc.vector.scalar_tensor_tensor(
                    u[:], uz[:], DECAY, v_t, Alu.mult, Alu.add)
                nc.vector.scalar_tensor_tensor(
                    mem1[:], z[:], DECAY, w[:], Alu.mult, Alu.add)
                nc.vector.tensor_scalar(
                    ns[:], mem1[:], THRESH, None, Alu.is_le)
                nc.scalar.dma_start(sp_d[t, :, sl], ns[:])

        g, j0 = 0, 0
        for j in range(JC):
            wj = wpool.tile([128, KS, 128], f32r, tag="wj", name=f"w{j}")
            nc.sync.dma_start(wj[:], wt_d[j])
            ps = ppool.tile([128, N], f32)
            for s in range(KS):
                nc.tensor.matmul(
                    ps[:], wj[:, s, :], x_tiles[s // KCH][:, s % KCH, :],
                    start=(s == 0), stop=(s == KS - 1))
            nc.scalar.copy(
                fc_sbuf[:, :, j * BL:(j + 1) * BL],
                ps.rearrange("p (t b) -> p t b", b=BL))
            if j == j0 + GROUPS[g] - 1:
                emit_recurrence(g, j0)
                j0 += GROUPS[g]
                g += 1
    _legalize_waits(nc, mybir)
    return nc


_CACHE = {}


def _get_runner():
    """Compile once; return (fn, in_names, out_names, zero_outs, mesh)."""
    if "fn" in _CACHE:
        return _CACHE["fn"]
    global LAST_NC
    import jax
    import numpy as _np
    from jax.sharding import Mesh, PartitionSpec
    from jax.experimental.shard_map import shard_map
    import concourse.mybir as mybir
    from concourse import bass2jax

    bass2jax.install_neuronx_cc_hook()
    nc = _build_bass()
    LAST_NC = nc

    in_names, out_names, out_avals, zero_outs = [], [], [], []
    partition_name = nc.partition_id_tensor.name if nc.partition_id_tensor else None
    for alloc in nc.m.functions[0].allocations:
        if not isinstance(alloc, mybir.MemoryLocationSet):
            continue
        name = alloc.memorylocations[0].name
        if alloc.kind == "ExternalInput":
            if name != partition_name:
                in_names.append(name)
        elif alloc.kind == "ExternalOutput":
            shape = tuple(alloc.tensor_shape)
            dtype = mybir.dt.np(alloc.dtype)
            out_names.append(name)
            out_avals.append(jax.core.ShapedArray(shape, dtype))
            zero_outs.append(_np.zeros(shape, dtype))
    n_params = len(in_names)
    all_in_names = list(in_names) + list(out_names)
    if partition_name is not None:
        all_in_names.append(partition_name)
    donate = tuple(range(n_params, n_params + len(out_names)))

    def _body(*args):
        operands = list(args)
        if partition_name is not None:
            operands.append(bass2jax.partition_id_tensor())
        outs = bass2jax._bass_exec_p.bind(
            *operands,
            out_avals=tuple(out_avals),
            in_names=tuple(all_in_names),
            out_names=tuple(out_names),
            lowering_input_output_aliases=(),
            sim_require_finite=True,
            sim_require_nnan=True,
            nc=nc,
        )
        return tuple(outs)

    devices = jax.devices()[:NCORES]
    mesh = Mesh(_np.asarray(devices), ("core",))
    n_all = n_params + len(out_names)
    sharded = jax.jit(
        shard_map(_body, mesh=mesh,
                  in_specs=(PartitionSpec("core"),) * n_all,
                  out_specs=(PartitionSpec("core"),) * len(out_names),
                  check_rep=False),
        donate_argnums=donate, keep_unused=True,
    )
    _CACHE["fn"] = (sharded, in_names, out_names, zero_outs, mesh)
    return _CACHE["fn"]


def kernel(x, W_fc, b_fc, W_lif, b_lif):
    global LAST_EXEC_S
    if np.any(b_fc != 0) or np.any(b_lif != 0):
        return _numpy_fallback(x, W_fc, b_fc, W_lif, b_lif)
    import time
    import jax

    Ws = float(W_lif[0, 0]) + float(W_lif[0, 1]) + float(W_lif[0, 2])
    # lhsT layout: wt[j, p, s, m] = (Ws*W_fc).T[s*128+p, j*128+m]
    Wt = np.ascontiguousarray((W_fc.astype(np.float32) * np.float32(Ws)).T)
    wt = np.ascontiguousarray(
        Wt.reshape(KS, 128, JC, 128).transpose(2, 1, 0, 3))

    per_core = {"wt": [], "xt": []}
    for c in range(NCORES):
        xs = np.ascontiguousarray(
            x[c * BL:(c + 1) * BL].astype(np.float32).transpose(2, 1, 0)
        ).reshape(IN, N)  # [IN, t*BL+b]
        per_core["xt"].append(np.ascontiguousarray(
            xs.reshape(KS, 128, N).transpose(1, 0, 2)))
        per_core["wt"].append(wt)

    sharded, in_names, out_names, zero_outs, mesh = _get_runner()
    concat_in = [np.concatenate(per_core[n], axis=0) for n in in_names]
    concat_zero = [np.concatenate([z] * NCORES, axis=0) for z in zero_outs]

    from jax.sharding import NamedSharding, PartitionSpec
    shd = NamedSharding(mesh, PartitionSpec("core"))
    args = [jax.device_put(a, shd) for a in concat_in + concat_zero]
    for a in args:
        a.block_until_ready()
    t0 = time.time()
    out_arrs = sharded(*args)
    jax.block_until_ready(out_arrs)
    LAST_EXEC_S = time.time() - t0
    out_arrs = [np.asarray(o) for o in out_arrs]

    sp_all = out_arrs[out_names.index("sp")]            # [8*T, 128, FREE]
    out = np.empty((B, T, C), dtype=np.float32)
    for c in range(NCORES):
        sp = sp_all[c * T:(c + 1) * T]                  # [T, 128, FREE]
        arr = sp.reshape(T, 128, JC, BL)                # (t, p, j, b)
        spikes = 1.0 - np.transpose(arr, (3, 0, 2, 1))  # (b, t, j, p)
        out[c * BL:(c + 1) * BL] = spikes.reshape(BL, T, C)
    return out
